# revision 2
# baseline (speedup 1.0000x reference)
"""DressedQuantumNet on 8 TRN2 NeuronCores (pure data parallel).

Math: pre-net angles th = X @ pre_w.T + pre_b.  The quantum circuit after
the batch-dependent RY(th) layer is a FIXED unitary V (it only depends on
q_weights), and the initial state is the product state
psi = kron_w [a_w, b_w] with a=(cos(th/2)-sin(th/2))/sqrt(2),
b=(cos(th/2)+sin(th/2))/sqrt(2) (real).  Hence

  <Z_w> = psi^T Re(V^H Z_w V) psi
  out_k = psi^T C_k psi + post_b_k,   C_k = sum_w post_w[k,w] Re(V^H Z_w V)

and since (u u^T) is affine in (sin th, cos th), the whole quadratic form
collapses to an 81-coefficient multilinear polynomial in
v_w = [1, sin th_w, cos th_w]:

  out_k = sum_{m in 3^4} T_k[m] * prod_w v_w[m_w]

T_k is precomputed on host (tiny), so the device only computes the big
[B,512]@[512,4] matmul, sin/cos, and a few batched elementwise products.

Device layout (per core, batch on SBUF partitions):
  - X shipped fp16 (host-rounded; rel err ~8e-4 vs the 2e-2 gate), slabs
    of 8 row-tiles DMA'd on alternating HWDGE rings (sync/scalar).
  - per tile: ones-row matmul seeds PSUM with [pre_b | pre_b + pi/2],
    then 4 fp16 matmuls vs duplicated W cols -> ang[p, t, 8] = theta and
    theta + pi/2 in one PSUM tile.
  - one conditional range-wrap into [-pi, pi] on DVE (|theta| < 4.6, so
    one wrap is exact), one Sin LUT pass -> sin/cos fp16.
  - kron products + T contraction in fp16 (2x DVE rate), outer products
    and the small sk multiply on the otherwise-idle Pool (gpsimd) engine.
"""

from contextlib import ExitStack

import numpy as np

import concourse.bass as bass
import concourse.bacc as bacc_mod
import concourse.mybir as mybir
from concourse.bass_utils import run_bass_kernel_spmd
from concourse.tile import TileContext

N_CORES = 8
B_TOTAL = 65536
F_IN = 512
ROWS = B_TOTAL // N_CORES   # 8192 rows per core
P = 128
N_TILES = ROWS // P         # 64 row-tiles
SLAB = 8                    # row-tiles per input DMA (1 MiB fp16)
SUPER = 2                   # slabs per quantum-stage group

F32 = mybir.dt.float32
FP16 = mybir.dt.float16
PI = float(np.pi)

N_QUBITS, VAR_DEPTH = 4, 3


# ----------------------------------------------------------------- host math
def _gate_1q(g, w):
    ops = [np.eye(2, dtype=complex)] * N_QUBITS
    ops[w] = g
    U = ops[0]
    for i in range(1, N_QUBITS):
        U = np.kron(U, ops[i])
    return U


def _bit(i, w):  # wire 0 = most significant
    return (i >> (N_QUBITS - 1 - w)) & 1


def _cnot(c, t):
    M = np.zeros((16, 16), dtype=complex)
    for i in range(16):
        j = i ^ (1 << (N_QUBITS - 1 - t)) if _bit(i, c) else i
        M[j, i] = 1.0
    return M


def _ry(theta):
    c, s = np.cos(theta / 2), np.sin(theta / 2)
    return np.array([[c, -s], [s, c]], dtype=complex)


def _rz(theta):
    ph = np.exp(1j * theta / 2)
    return np.array([[np.conj(ph), 0], [0, ph]], dtype=complex)


def _fixed_unitary(qw):
    V = np.eye(16, dtype=complex)

    def app(Gm):
        nonlocal V
        V = Gm @ V

    def entangle():
        app(_cnot(0, 1)); app(_cnot(2, 3)); app(_cnot(1, 2))

    for k in range(VAR_DEPTH):
        entangle()
        for w in range(N_QUBITS):
            app(_gate_1q(_ry(qw[k, w]), w))
        for w in range(N_QUBITS):
            app(_gate_1q(_rz(qw[k, w]), w))
    for k in range(VAR_DEPTH):
        entangle()
        for w in range(N_QUBITS):
            app(_gate_1q(_ry(qw[k, w]), w))
        for w in range(N_QUBITS):
            app(_gate_1q(_rz(qw[3 + k, w]), w))
    entangle()
    return V


def _build_T(q_weights, post_w, post_b):
    """[2, 81] coefficients; post_b folded into the constant term."""
    V = _fixed_unitary(np.asarray(q_weights, dtype=np.float64))
    E = np.zeros((3, 2, 2))
    E[0] = [[0.5, 0.0], [0.0, 0.5]]
    E[1] = [[-0.5, 0.0], [0.0, 0.5]]
    E[2] = [[0.0, 0.5], [0.5, 0.0]]
    Ts = []
    for k in range(2):
        C = np.zeros((16, 16), dtype=complex)
        for w in range(N_QUBITS):
            z = np.array([1.0 - 2.0 * _bit(i, w) for i in range(16)])
            C += post_w[k, w] * (V.conj().T @ np.diag(z) @ V)
        A = C.real.reshape([2] * 8)
        T = np.einsum("abcdefgh,iae,jbf,kcg,ldh->ijkl", A, E, E, E, E)
        T = T.reshape(81).copy()
        T[0] += post_b[k]
        Ts.append(T)
    return np.stack(Ts).astype(np.float32)  # [2, 81]


# ------------------------------------------------------------- device kernel
def build_bass(rows=ROWS):
    n_tiles = rows // P
    n_slabs = n_tiles // SLAB
    assert n_tiles % SLAB == 0

    nc = bacc_mod.Bacc(None, target_bir_lowering=False)
    # host-packed flat: per slab [P, 4, SLAB*P], pack[p,k,b] = X[b, 128k+p]
    x_d = nc.dram_tensor("xtp", [rows * 4 * P], FP16, kind="ExternalInput")
    w_d = nc.dram_tensor("wdup", [P, 32], FP16, kind="ExternalInput")
    pb_d = nc.dram_tensor("pb8", [1, 8], F32, kind="ExternalInput")
    tc_d = nc.dram_tensor("tcoef", [P, 162], FP16, kind="ExternalInput")
    # out_dev[p, t, k] = out[t*128 + p, k]; host unscrambles
    out_d = nc.dram_tensor("out", [P, n_tiles, 2], F32, kind="ExternalOutput")

    with TileContext(nc) as tc, ExitStack() as ctx:
        const = ctx.enter_context(tc.tile_pool(name="const", bufs=1))
        wdup = const.tile([P, 32], FP16)
        nc.scalar.dma_start(wdup, w_d[:])
        pb8 = const.tile([1, 8], F32)
        nc.scalar.dma_start(pb8, pb_d[:])
        tco = const.tile([P, 162], FP16)
        nc.scalar.dma_start(tco, tc_d[:])
        ones = const.tile([1, P], F32)
        nc.vector.memset(ones, 1.0)
        zt = const.tile([P, 1], F32)
        nc.vector.memset(zt, 0.0)
        warm = const.tile([P, 1], F32)
        # preload the Sin table set under the first input DMA
        nc.scalar.activation(warm, zt, mybir.ActivationFunctionType.Sin)

        xp = ctx.enter_context(tc.tile_pool(name="xin", bufs=6))
        angp = ctx.enter_context(tc.tile_pool(name="ang", bufs=3, space="PSUM"))
        thp = ctx.enter_context(tc.tile_pool(name="th", bufs=2))
        vvp = ctx.enter_context(tc.tile_pool(name="vv", bufs=2))
        wp = ctx.enter_context(tc.tile_pool(name="wpair", bufs=2))
        tqp = ctx.enter_context(tc.tile_pool(name="tq", bufs=2))
        qkp = ctx.enter_context(tc.tile_pool(name="qk", bufs=2))
        rp = ctx.enter_context(tc.tile_pool(name="res", bufs=2))

        dma_engines = [nc.sync, nc.scalar]
        slab_idx = 0
        off = 0
        for s0 in range(0, n_slabs, SUPER):
            sl_cnt = min(SUPER, n_slabs - s0)
            sgt = sl_cnt * SLAB
            x_tiles = []
            for _ in range(sl_cnt):
                gb = SLAB * P
                base = slab_idx * P * 4 * gb
                xt = xp.tile([P, 4, gb], FP16, tag="x")
                dma_engines[slab_idx % 2].dma_start(
                    xt,
                    x_d[base:base + P * 4 * gb].rearrange(
                        "(p k b) -> p k b", p=P, k=4),
                )
                x_tiles.append(xt)
                slab_idx += 1

            # ang[p, t, 0:4] = theta + pre_b; ang[p, t, 4:8] = theta + pre_b + pi/2
            ang = angp.tile([P, sgt, 8], F32)
            for ti in range(sgt):
                sl, tloc = divmod(ti, SLAB)
                bs = tloc * P
                nc.tensor.matmul(
                    ang[:, ti, :], ones[:, :], pb8[:, :],
                    start=True, stop=False,
                )
                for k in range(4):
                    nc.tensor.matmul(
                        ang[:, ti, :],
                        x_tiles[sl][:, k, bs:bs + P],
                        wdup[:, 8 * k:8 * k + 8],
                        start=False, stop=(k == 3),
                    )

            # range-wrap into [-pi, pi] (|theta| < 4.6, one wrap is exact;
            # the cos half only ever needs the positive wrap)
            m1 = thp.tile([P, sgt, 8], F32, tag="m1")
            nc.vector.tensor_scalar(
                m1, ang, PI, -2.0 * PI,
                op0=mybir.AluOpType.is_gt, op1=mybir.AluOpType.mult,
            )
            a1 = thp.tile([P, sgt, 8], F32, tag="a1")
            nc.vector.tensor_add(a1, ang, m1)
            m2 = thp.tile([P, sgt, 8], F32, tag="m2")
            nc.vector.tensor_scalar(
                m2, a1, -PI, 2.0 * PI,
                op0=mybir.AluOpType.is_lt, op1=mybir.AluOpType.mult,
            )
            thin = thp.tile([P, sgt, 8], FP16, tag="thin")
            nc.vector.tensor_add(thin, a1, m2)

            # v = [1, sin, cos] per wire: vv[p, g, m, w]
            vv = vvp.tile([P, sgt, 3, 4], FP16)
            nc.gpsimd.memset(vv[:, :, 0, :], 1.0)
            nc.scalar.activation(
                vv[:, :, 1:3, :],
                thin.rearrange("p g (c w) -> p g c w", c=2),
                mybir.ActivationFunctionType.Sin,
            )

            # w01[m0,m1] = v0[m0]*v1[m1]; w23[m2,m3] = v2[m2]*v3[m3]
            wpair = wp.tile([P, sgt, 2, 3, 3], FP16)
            nc.gpsimd.tensor_mul(
                wpair[:, :, 0],
                vv[:, :, :, 0].unsqueeze(3).broadcast_to([P, sgt, 3, 3]),
                vv[:, :, :, 1].unsqueeze(2).broadcast_to([P, sgt, 3, 3]),
            )
            nc.gpsimd.tensor_mul(
                wpair[:, :, 1],
                vv[:, :, :, 2].unsqueeze(3).broadcast_to([P, sgt, 3, 3]),
                vv[:, :, :, 3].unsqueeze(2).broadcast_to([P, sgt, 3, 3]),
            )
            w01 = wpair[:, :, 0].rearrange("p g a b -> p g (a b)")
            w23 = wpair[:, :, 1].rearrange("p g a b -> p g (a b)")

            # tq[p,g,km,m23] = w23[m23] * T[km, m23]  ((k,m01) merged -> 18)
            tq = tqp.tile([P, sgt, 18, 9], FP16)
            nc.vector.tensor_mul(
                tq,
                w23.unsqueeze(2).broadcast_to([P, sgt, 18, 9]),
                tco[:, 0:162].rearrange("p (km b) -> p km b", b=9)
                   .unsqueeze(1).broadcast_to([P, sgt, 18, 9]),
            )
            qk = qkp.tile([P, sgt, 18], FP16, tag="qk")
            with nc.allow_low_precision(reason="fp16 qk; gate is 2e-2"):
                nc.vector.tensor_reduce(
                    qk, tq, axis=mybir.AxisListType.X, op=mybir.AluOpType.add
                )
            sk = qkp.tile([P, sgt, 2, 9], F32, tag="sk")
            nc.gpsimd.tensor_mul(
                sk,
                qk.rearrange("p g (k m) -> p g k m", m=9),
                w01.unsqueeze(2).broadcast_to([P, sgt, 2, 9]),
            )
            res = rp.tile([P, sgt, 2], F32)
            nc.vector.tensor_reduce(
                res, sk, axis=mybir.AxisListType.X, op=mybir.AluOpType.add
            )
            nc.sync.dma_start(out_d[:, off:off + sgt, :], res)
            off += sgt

    nc.finalize()
    return nc


_NC_CACHE = {}


def _get_nc(rows=ROWS):
    if rows not in _NC_CACHE:
        _NC_CACHE[rows] = build_bass(rows=rows)
    return _NC_CACHE[rows]


def _host_consts(pre_w, pre_b, q_weights, post_w, post_b):
    pre_w = np.asarray(pre_w, dtype=np.float32)
    wdup = np.zeros((P, 32), dtype=np.float16)
    for k in range(4):
        blk = pre_w.T[P * k:P * (k + 1)].astype(np.float16)  # [128, 4]
        wdup[:, 8 * k:8 * k + 4] = blk
        wdup[:, 8 * k + 4:8 * k + 8] = blk
    pb = np.asarray(pre_b, np.float64)
    pb8 = np.concatenate([pb, pb + 0.5 * np.pi]).reshape(1, 8).astype(np.float32)
    T = _build_T(
        np.asarray(q_weights, np.float64),
        np.asarray(post_w, np.float64),
        np.asarray(post_b, np.float64),
    )  # [2, 81] f32
    tco = np.broadcast_to(T.reshape(162), (P, 162)).astype(np.float16).copy()
    return {
        "wdup": np.ascontiguousarray(wdup),
        "pb8": np.ascontiguousarray(pb8),
        "tcoef": np.ascontiguousarray(tco),
    }


def _pack_x(x):
    """x [ROWS, F] f32 -> flat fp16, per-slab [P, 4, SLAB*P] packs with
    pack[p, k, b] = x[slab_row0 + b, 128*k + p]."""
    rows = x.shape[0]
    h = x.astype(np.float16)
    gb = SLAB * P
    parts = []
    for r0 in range(0, rows, gb):
        blk = h[r0:r0 + gb].reshape(gb, 4, P).transpose(2, 1, 0)
        parts.append(np.ascontiguousarray(blk).reshape(-1))
    return np.concatenate(parts)


def run(input_features, pre_w, pre_b, q_weights, post_w, post_b, **spmd_kwargs):
    x = np.asarray(input_features, dtype=np.float32)
    assert x.shape == (B_TOTAL, F_IN), x.shape
    consts = _host_consts(pre_w, pre_b, q_weights, post_w, post_b)
    in_maps = []
    for c in range(N_CORES):
        in_maps.append(dict(consts, xtp=_pack_x(x[c * ROWS:(c + 1) * ROWS])))
    nc = _get_nc()
    r = run_bass_kernel_spmd(nc, in_maps, core_ids=list(range(N_CORES)), **spmd_kwargs)
    # out_dev[p, t, k] -> out[t*128 + p, k]
    out = np.concatenate(
        [r.results[c]["out"].transpose(1, 0, 2).reshape(ROWS, 2) for c in range(N_CORES)],
        axis=0,
    )
    return out.astype(np.float32), r


def kernel(input_features, pre_w, pre_b, q_weights, post_w, post_b):
    out, _ = run(input_features, pre_w, pre_b, q_weights, post_w, post_b)
    return out


# revision 3
# speedup vs baseline: 1.1889x; 1.1889x over previous
"""DressedQuantumNet on 8 TRN2 NeuronCores (pure data parallel).

Math: pre-net angles th = X @ pre_w.T + pre_b.  The quantum circuit after
the batch-dependent RY(th) layer is a FIXED unitary V (it only depends on
q_weights), and the initial state is the product state
psi = kron_w [a_w, b_w] with a=(cos(th/2)-sin(th/2))/sqrt(2),
b=(cos(th/2)+sin(th/2))/sqrt(2) (real).  Hence

  <Z_w> = psi^T Re(V^H Z_w V) psi
  out_k = psi^T C_k psi + post_b_k,   C_k = sum_w post_w[k,w] Re(V^H Z_w V)

and since (u u^T) is affine in (sin th, cos th), the whole quadratic form
collapses to an 81-coefficient multilinear polynomial in
v_w = [1, sin th_w, cos th_w]:

  out_k = sum_{m in 3^4} T_k[m] * prod_w v_w[m_w]

T_k is precomputed on host (tiny), so the device only computes the big
[B,512]@[512,4] matmul, sin/cos, and a few batched elementwise products.

Device layout (per core, batch on SBUF partitions):
  - X shipped fp16 (host-rounded; rel err ~8e-4 vs the 2e-2 gate), slabs
    of 8 row-tiles DMA'd on alternating HWDGE rings (sync/scalar).
  - per tile: ones-row matmul seeds PSUM with [pre_b | pre_b + pi/2],
    then 4 fp16 matmuls vs duplicated W cols -> ang[p, t, 8] = theta and
    theta + pi/2 in one PSUM tile.
  - one conditional range-wrap into [-pi, pi] on DVE (|theta| < 4.6, so
    one wrap is exact), one Sin LUT pass -> sin/cos fp16.
  - kron products + T contraction in fp16 (2x DVE rate), outer products
    and the small sk multiply on the otherwise-idle Pool (gpsimd) engine.
"""

from contextlib import ExitStack

import numpy as np

import concourse.bass as bass
import concourse.bacc as bacc_mod
import concourse.mybir as mybir
from concourse.bass_utils import run_bass_kernel_spmd
from concourse.tile import TileContext

N_CORES = 8
B_TOTAL = 65536
F_IN = 512
ROWS = B_TOTAL // N_CORES   # 8192 rows per core
P = 128
N_TILES = ROWS // P         # 64 row-tiles
SLAB = 8                    # row-tiles per input DMA (1 MiB fp16)
SUPER = 2                   # slabs per quantum-stage group

F32 = mybir.dt.float32
FP16 = mybir.dt.float16
PI = float(np.pi)

N_QUBITS, VAR_DEPTH = 4, 3


# ----------------------------------------------------------------- host math
def _gate_1q(g, w):
    ops = [np.eye(2, dtype=complex)] * N_QUBITS
    ops[w] = g
    U = ops[0]
    for i in range(1, N_QUBITS):
        U = np.kron(U, ops[i])
    return U


def _bit(i, w):  # wire 0 = most significant
    return (i >> (N_QUBITS - 1 - w)) & 1


def _cnot(c, t):
    M = np.zeros((16, 16), dtype=complex)
    for i in range(16):
        j = i ^ (1 << (N_QUBITS - 1 - t)) if _bit(i, c) else i
        M[j, i] = 1.0
    return M


def _ry(theta):
    c, s = np.cos(theta / 2), np.sin(theta / 2)
    return np.array([[c, -s], [s, c]], dtype=complex)


def _rz(theta):
    ph = np.exp(1j * theta / 2)
    return np.array([[np.conj(ph), 0], [0, ph]], dtype=complex)


def _fixed_unitary(qw):
    V = np.eye(16, dtype=complex)

    def app(Gm):
        nonlocal V
        V = Gm @ V

    def entangle():
        app(_cnot(0, 1)); app(_cnot(2, 3)); app(_cnot(1, 2))

    for k in range(VAR_DEPTH):
        entangle()
        for w in range(N_QUBITS):
            app(_gate_1q(_ry(qw[k, w]), w))
        for w in range(N_QUBITS):
            app(_gate_1q(_rz(qw[k, w]), w))
    for k in range(VAR_DEPTH):
        entangle()
        for w in range(N_QUBITS):
            app(_gate_1q(_ry(qw[k, w]), w))
        for w in range(N_QUBITS):
            app(_gate_1q(_rz(qw[3 + k, w]), w))
    entangle()
    return V


def _build_T(q_weights, post_w, post_b):
    """[2, 81] coefficients; post_b folded into the constant term."""
    V = _fixed_unitary(np.asarray(q_weights, dtype=np.float64))
    E = np.zeros((3, 2, 2))
    E[0] = [[0.5, 0.0], [0.0, 0.5]]
    E[1] = [[-0.5, 0.0], [0.0, 0.5]]
    E[2] = [[0.0, 0.5], [0.5, 0.0]]
    Ts = []
    for k in range(2):
        C = np.zeros((16, 16), dtype=complex)
        for w in range(N_QUBITS):
            z = np.array([1.0 - 2.0 * _bit(i, w) for i in range(16)])
            C += post_w[k, w] * (V.conj().T @ np.diag(z) @ V)
        A = C.real.reshape([2] * 8)
        T = np.einsum("abcdefgh,iae,jbf,kcg,ldh->ijkl", A, E, E, E, E)
        T = T.reshape(81).copy()
        T[0] += post_b[k]
        Ts.append(T)
    return np.stack(Ts).astype(np.float32)  # [2, 81]


# ------------------------------------------------------------- device kernel
def build_bass(rows=ROWS):
    n_tiles = rows // P
    n_slabs = n_tiles // SLAB
    assert n_tiles % SLAB == 0

    nc = bacc_mod.Bacc(None, target_bir_lowering=False)
    # host-packed flat: per slab [P, 4, SLAB*P], pack[p,k,b] = X[b, 128k+p]
    x_d = nc.dram_tensor("xtp", [rows * 4 * P], FP16, kind="ExternalInput")
    w_d = nc.dram_tensor("wdup", [P, 32], FP16, kind="ExternalInput")
    pb_d = nc.dram_tensor("pb8", [1, 8], FP16, kind="ExternalInput")
    tc_d = nc.dram_tensor("tcoef", [P, 162], FP16, kind="ExternalInput")
    # out_dev[p, t, k] = out[t*128 + p, k]; host unscrambles
    out_d = nc.dram_tensor("out", [P, n_tiles, 2], F32, kind="ExternalOutput")

    with TileContext(nc) as tc, ExitStack() as ctx:
        const = ctx.enter_context(tc.tile_pool(name="const", bufs=1))
        wdup = const.tile([P, 32], FP16)
        nc.scalar.dma_start(wdup, w_d[:])
        pb8 = const.tile([1, 8], FP16)
        nc.scalar.dma_start(pb8, pb_d[:])
        tco = const.tile([P, 162], FP16)
        nc.scalar.dma_start(tco, tc_d[:])
        ones = const.tile([1, P], FP16)
        nc.vector.memset(ones, 1.0)
        zt = const.tile([P, 1], F32)
        nc.vector.memset(zt, 0.0)
        warm = const.tile([P, 1], F32)
        # preload the Sin table set under the first input DMA
        nc.scalar.activation(warm, zt, mybir.ActivationFunctionType.Sin)

        xp = ctx.enter_context(tc.tile_pool(name="xin", bufs=6))
        angp = ctx.enter_context(tc.tile_pool(name="ang", bufs=3, space="PSUM"))
        thp = ctx.enter_context(tc.tile_pool(name="th", bufs=2))
        vvp = ctx.enter_context(tc.tile_pool(name="vv", bufs=2))
        wp = ctx.enter_context(tc.tile_pool(name="wpair", bufs=2))
        tqp = ctx.enter_context(tc.tile_pool(name="tq", bufs=2))
        qkp = ctx.enter_context(tc.tile_pool(name="qk", bufs=2))
        rp = ctx.enter_context(tc.tile_pool(name="res", bufs=2))

        dma_engines = [nc.sync, nc.scalar]
        slab_idx = 0
        off = 0
        for s0 in range(0, n_slabs, SUPER):
            sl_cnt = min(SUPER, n_slabs - s0)
            sgt = sl_cnt * SLAB
            x_tiles = []
            for _ in range(sl_cnt):
                gb = SLAB * P
                base = slab_idx * P * 4 * gb
                xt = xp.tile([P, 4, gb], FP16, tag="x")
                dma_engines[slab_idx % 2].dma_start(
                    xt,
                    x_d[base:base + P * 4 * gb].rearrange(
                        "(p k b) -> p k b", p=P, k=4),
                )
                x_tiles.append(xt)
                slab_idx += 1

            # ang[p, t, 0:4] = theta + pre_b; ang[p, t, 4:8] = theta + pre_b + pi/2
            ang = angp.tile([P, sgt, 8], F32)
            for ti in range(sgt):
                sl, tloc = divmod(ti, SLAB)
                bs = tloc * P
                nc.tensor.matmul(
                    ang[:, ti, :], ones[:, :], pb8[:, :],
                    start=True, stop=False,
                )
                for k in range(4):
                    nc.tensor.matmul(
                        ang[:, ti, :],
                        x_tiles[sl][:, k, bs:bs + P],
                        wdup[:, 8 * k:8 * k + 8],
                        start=False, stop=(k == 3),
                    )

            # range-wrap into [-pi, pi] (|theta| < 4.6, one wrap is exact;
            # the cos half only ever needs the positive wrap)
            m1 = thp.tile([P, sgt, 8], F32, tag="m1")
            nc.vector.tensor_scalar(
                m1, ang, PI, -2.0 * PI,
                op0=mybir.AluOpType.is_gt, op1=mybir.AluOpType.mult,
            )
            a1 = thp.tile([P, sgt, 8], F32, tag="a1")
            nc.vector.tensor_add(a1, ang, m1)
            m2 = thp.tile([P, sgt, 8], F32, tag="m2")
            nc.vector.tensor_scalar(
                m2, a1, -PI, 2.0 * PI,
                op0=mybir.AluOpType.is_lt, op1=mybir.AluOpType.mult,
            )
            thin = thp.tile([P, sgt, 8], FP16, tag="thin")
            nc.vector.tensor_add(thin, a1, m2)

            # v = [1, sin, cos] per wire: vv[p, g, m, w]
            vv = vvp.tile([P, sgt, 3, 4], FP16)
            nc.gpsimd.memset(vv[:, :, 0, :], 1.0)
            nc.scalar.activation(
                vv[:, :, 1:3, :],
                thin.rearrange("p g (c w) -> p g c w", c=2),
                mybir.ActivationFunctionType.Sin,
            )

            # w01[m0,m1] = v0[m0]*v1[m1]; w23[m2,m3] = v2[m2]*v3[m3]
            wpair = wp.tile([P, sgt, 2, 3, 3], FP16)
            nc.gpsimd.tensor_mul(
                wpair[:, :, 0],
                vv[:, :, :, 0].unsqueeze(3).broadcast_to([P, sgt, 3, 3]),
                vv[:, :, :, 1].unsqueeze(2).broadcast_to([P, sgt, 3, 3]),
            )
            nc.gpsimd.tensor_mul(
                wpair[:, :, 1],
                vv[:, :, :, 2].unsqueeze(3).broadcast_to([P, sgt, 3, 3]),
                vv[:, :, :, 3].unsqueeze(2).broadcast_to([P, sgt, 3, 3]),
            )
            w01 = wpair[:, :, 0].rearrange("p g a b -> p g (a b)")
            w23 = wpair[:, :, 1].rearrange("p g a b -> p g (a b)")

            # tq[p,g,km,m23] = w23[m23] * T[km, m23]  ((k,m01) merged -> 18)
            tq = tqp.tile([P, sgt, 18, 9], FP16)
            nc.vector.tensor_mul(
                tq,
                w23.unsqueeze(2).broadcast_to([P, sgt, 18, 9]),
                tco[:, 0:162].rearrange("p (km b) -> p km b", b=9)
                   .unsqueeze(1).broadcast_to([P, sgt, 18, 9]),
            )
            qk = qkp.tile([P, sgt, 18], FP16, tag="qk")
            with nc.allow_low_precision(reason="fp16 qk; gate is 2e-2"):
                nc.vector.tensor_reduce(
                    qk, tq, axis=mybir.AxisListType.X, op=mybir.AluOpType.add
                )
            sk = qkp.tile([P, sgt, 2, 9], F32, tag="sk")
            nc.gpsimd.tensor_mul(
                sk,
                qk.rearrange("p g (k m) -> p g k m", m=9),
                w01.unsqueeze(2).broadcast_to([P, sgt, 2, 9]),
            )
            res = rp.tile([P, sgt, 2], F32)
            nc.vector.tensor_reduce(
                res, sk, axis=mybir.AxisListType.X, op=mybir.AluOpType.add
            )
            nc.sync.dma_start(out_d[:, off:off + sgt, :], res)
            off += sgt

    nc.finalize()
    return nc


_NC_CACHE = {}


def _get_nc(rows=ROWS):
    if rows not in _NC_CACHE:
        _NC_CACHE[rows] = build_bass(rows=rows)
    return _NC_CACHE[rows]


def _host_consts(pre_w, pre_b, q_weights, post_w, post_b):
    pre_w = np.asarray(pre_w, dtype=np.float32)
    wdup = np.zeros((P, 32), dtype=np.float16)
    for k in range(4):
        blk = pre_w.T[P * k:P * (k + 1)].astype(np.float16)  # [128, 4]
        wdup[:, 8 * k:8 * k + 4] = blk
        wdup[:, 8 * k + 4:8 * k + 8] = blk
    pb = np.asarray(pre_b, np.float64)
    pb8 = np.concatenate([pb, pb + 0.5 * np.pi]).reshape(1, 8).astype(np.float16)
    T = _build_T(
        np.asarray(q_weights, np.float64),
        np.asarray(post_w, np.float64),
        np.asarray(post_b, np.float64),
    )  # [2, 81] f32
    tco = np.broadcast_to(T.reshape(162), (P, 162)).astype(np.float16).copy()
    return {
        "wdup": np.ascontiguousarray(wdup),
        "pb8": np.ascontiguousarray(pb8),
        "tcoef": np.ascontiguousarray(tco),
    }


def _pack_x(x):
    """x [ROWS, F] f32 -> flat fp16, per-slab [P, 4, SLAB*P] packs with
    pack[p, k, b] = x[slab_row0 + b, 128*k + p]."""
    rows = x.shape[0]
    h = x.astype(np.float16)
    gb = SLAB * P
    parts = []
    for r0 in range(0, rows, gb):
        blk = h[r0:r0 + gb].reshape(gb, 4, P).transpose(2, 1, 0)
        parts.append(np.ascontiguousarray(blk).reshape(-1))
    return np.concatenate(parts)


def run(input_features, pre_w, pre_b, q_weights, post_w, post_b, **spmd_kwargs):
    x = np.asarray(input_features, dtype=np.float32)
    assert x.shape == (B_TOTAL, F_IN), x.shape
    consts = _host_consts(pre_w, pre_b, q_weights, post_w, post_b)
    in_maps = []
    for c in range(N_CORES):
        in_maps.append(dict(consts, xtp=_pack_x(x[c * ROWS:(c + 1) * ROWS])))
    nc = _get_nc()
    r = run_bass_kernel_spmd(nc, in_maps, core_ids=list(range(N_CORES)), **spmd_kwargs)
    # out_dev[p, t, k] -> out[t*128 + p, k]
    out = np.concatenate(
        [r.results[c]["out"].transpose(1, 0, 2).reshape(ROWS, 2) for c in range(N_CORES)],
        axis=0,
    )
    return out.astype(np.float32), r


def kernel(input_features, pre_w, pre_b, q_weights, post_w, post_b):
    out, _ = run(input_features, pre_w, pre_b, q_weights, post_w, post_b)
    return out


# revision 4
# speedup vs baseline: 1.3647x; 1.1479x over previous
"""DressedQuantumNet on 8 TRN2 NeuronCores (pure data parallel).

Math: pre-net angles th = X @ pre_w.T + pre_b.  The quantum circuit after
the batch-dependent RY(th) layer is a FIXED unitary V (it only depends on
q_weights), and the initial state is the product state
psi = kron_w [a_w, b_w] with a=(cos(th/2)-sin(th/2))/sqrt(2),
b=(cos(th/2)+sin(th/2))/sqrt(2) (real).  Hence

  <Z_w> = psi^T Re(V^H Z_w V) psi
  out_k = psi^T C_k psi + post_b_k,   C_k = sum_w post_w[k,w] Re(V^H Z_w V)

and since (u u^T) is affine in (sin th, cos th), the whole quadratic form
collapses to an 81-coefficient multilinear polynomial in
v_w = [1, sin th_w, cos th_w]:

  out_k = sum_{m in 3^4} T_k[m] * prod_w v_w[m_w]

T_k is precomputed on host (tiny), so the device only computes the big
[B,512]@[512,4] matmul, sin/cos, and a few batched elementwise products.

Device layout (per core, batch on SBUF partitions):
  - X shipped fp16 (host-rounded; rel err ~8e-4 vs the 2e-2 gate), slabs
    of 8 row-tiles DMA'd on alternating HWDGE rings (sync/scalar).
  - per tile: ones-row matmul seeds PSUM with [pre_b | pre_b + pi/2],
    then 4 fp16 matmuls vs duplicated W cols -> ang[p, t, 8] = theta and
    theta + pi/2 in one PSUM tile.
  - one conditional range-wrap into [-pi, pi] on DVE (|theta| < 4.6, so
    one wrap is exact), one Sin LUT pass -> sin/cos fp16.
  - kron products + T contraction in fp16 (2x DVE rate), outer products
    and the small sk multiply on the otherwise-idle Pool (gpsimd) engine.
"""

from contextlib import ExitStack

import numpy as np

import concourse.bass as bass
import concourse.bacc as bacc_mod
import concourse.mybir as mybir
from concourse.bass_utils import run_bass_kernel_spmd
from concourse.tile import TileContext

N_CORES = 8
B_TOTAL = 65536
F_IN = 512
ROWS = B_TOTAL // N_CORES   # 8192 rows per core
P = 128
N_TILES = ROWS // P         # 64 row-tiles
SLAB = 8                    # row-tiles per input DMA (1 MiB fp16)
SUPER = 2                   # slabs per quantum-stage group

F32 = mybir.dt.float32
FP16 = mybir.dt.float16
PI = float(np.pi)

N_QUBITS, VAR_DEPTH = 4, 3


# ----------------------------------------------------------------- host math
def _gate_1q(g, w):
    ops = [np.eye(2, dtype=complex)] * N_QUBITS
    ops[w] = g
    U = ops[0]
    for i in range(1, N_QUBITS):
        U = np.kron(U, ops[i])
    return U


def _bit(i, w):  # wire 0 = most significant
    return (i >> (N_QUBITS - 1 - w)) & 1


def _cnot(c, t):
    M = np.zeros((16, 16), dtype=complex)
    for i in range(16):
        j = i ^ (1 << (N_QUBITS - 1 - t)) if _bit(i, c) else i
        M[j, i] = 1.0
    return M


def _ry(theta):
    c, s = np.cos(theta / 2), np.sin(theta / 2)
    return np.array([[c, -s], [s, c]], dtype=complex)


def _rz(theta):
    ph = np.exp(1j * theta / 2)
    return np.array([[np.conj(ph), 0], [0, ph]], dtype=complex)


def _fixed_unitary(qw):
    V = np.eye(16, dtype=complex)

    def app(Gm):
        nonlocal V
        V = Gm @ V

    def entangle():
        app(_cnot(0, 1)); app(_cnot(2, 3)); app(_cnot(1, 2))

    for k in range(VAR_DEPTH):
        entangle()
        for w in range(N_QUBITS):
            app(_gate_1q(_ry(qw[k, w]), w))
        for w in range(N_QUBITS):
            app(_gate_1q(_rz(qw[k, w]), w))
    for k in range(VAR_DEPTH):
        entangle()
        for w in range(N_QUBITS):
            app(_gate_1q(_ry(qw[k, w]), w))
        for w in range(N_QUBITS):
            app(_gate_1q(_rz(qw[3 + k, w]), w))
    entangle()
    return V


def _build_T(q_weights, post_w, post_b):
    """[2, 81] coefficients; post_b folded into the constant term."""
    V = _fixed_unitary(np.asarray(q_weights, dtype=np.float64))
    E = np.zeros((3, 2, 2))
    E[0] = [[0.5, 0.0], [0.0, 0.5]]
    E[1] = [[-0.5, 0.0], [0.0, 0.5]]
    E[2] = [[0.0, 0.5], [0.5, 0.0]]
    Ts = []
    for k in range(2):
        C = np.zeros((16, 16), dtype=complex)
        for w in range(N_QUBITS):
            z = np.array([1.0 - 2.0 * _bit(i, w) for i in range(16)])
            C += post_w[k, w] * (V.conj().T @ np.diag(z) @ V)
        A = C.real.reshape([2] * 8)
        T = np.einsum("abcdefgh,iae,jbf,kcg,ldh->ijkl", A, E, E, E, E)
        T = T.reshape(81).copy()
        T[0] += post_b[k]
        Ts.append(T)
    return np.stack(Ts).astype(np.float32)  # [2, 81]


# ------------------------------------------------------------- device kernel
def build_bass(rows=ROWS):
    n_tiles = rows // P
    n_slabs = n_tiles // SLAB
    assert n_tiles % SLAB == 0

    nc = bacc_mod.Bacc(None, target_bir_lowering=False)
    # host-packed flat: per slab [P, 4, SLAB*P], pack[p,k,b] = X[b, 128k+p]
    x_d = nc.dram_tensor("xtp", [rows * 4 * P], FP16, kind="ExternalInput")
    w_d = nc.dram_tensor("wdup", [P, 32], FP16, kind="ExternalInput")
    pb_d = nc.dram_tensor("bias2", [P, 8], F32, kind="ExternalInput")
    tc_d = nc.dram_tensor("tcoef", [P, 162], FP16, kind="ExternalInput")
    # out_dev[p, t, k] = out[t*128 + p, k]; host unscrambles
    out_d = nc.dram_tensor("out", [P, n_tiles, 2], F32, kind="ExternalOutput")

    with TileContext(nc) as tc, ExitStack() as ctx:
        const = ctx.enter_context(tc.tile_pool(name="const", bufs=1))
        wdup = const.tile([P, 32], FP16)
        nc.scalar.dma_start(wdup, w_d[:])
        bia = const.tile([P, 8], F32)
        nc.scalar.dma_start(bia, pb_d[:])
        tco = const.tile([P, 162], FP16)
        nc.scalar.dma_start(tco, tc_d[:])
        zt = const.tile([P, 1], F32)
        nc.vector.memset(zt, 0.0)
        warm = const.tile([P, 1], F32)
        # preload the Sin table set under the first input DMA
        nc.scalar.activation(warm, zt, mybir.ActivationFunctionType.Sin)

        xp = ctx.enter_context(tc.tile_pool(name="xin", bufs=6))
        angp = ctx.enter_context(tc.tile_pool(name="ang", bufs=3, space="PSUM"))
        thp = ctx.enter_context(tc.tile_pool(name="th", bufs=3))
        vvp = ctx.enter_context(tc.tile_pool(name="vv", bufs=3))
        wp = ctx.enter_context(tc.tile_pool(name="wpair", bufs=3))
        tqp = ctx.enter_context(tc.tile_pool(name="tq", bufs=3))
        qkp = ctx.enter_context(tc.tile_pool(name="qk", bufs=3))
        rp = ctx.enter_context(tc.tile_pool(name="res", bufs=2))

        dma_engines = [nc.sync, nc.scalar]
        slab_idx = 0
        off = 0
        for s0 in range(0, n_slabs, SUPER):
            sl_cnt = min(SUPER, n_slabs - s0)
            sgt = sl_cnt * SLAB
            x_tiles = []
            for _ in range(sl_cnt):
                gb = SLAB * P
                base = slab_idx * P * 4 * gb
                xt = xp.tile([P, 4, gb], FP16, tag="x")
                dma_engines[slab_idx % 2].dma_start(
                    xt,
                    x_d[base:base + P * 4 * gb].rearrange(
                        "(p k b) -> p k b", p=P, k=4),
                )
                x_tiles.append(xt)
                slab_idx += 1

            # ang[p, t, 0:4] = theta + pre_b; ang[p, t, 4:8] = theta + pre_b + pi/2
            ang = angp.tile([P, sgt, 8], F32)
            for ti in range(sgt):
                sl, tloc = divmod(ti, SLAB)
                bs = tloc * P
                for k in range(4):
                    nc.tensor.matmul(
                        ang[:, ti, :],
                        x_tiles[sl][:, k, bs:bs + P],
                        wdup[:, 8 * k:8 * k + 8],
                        start=(k == 0), stop=(k == 3),
                    )

            # range-wrap into [-pi, pi] (|theta| < 4.6, one wrap is exact;
            # the cos half only ever needs the positive wrap)
            a0 = thp.tile([P, sgt, 8], F32, tag="a0")
            nc.vector.tensor_add(
                a0, ang,
                bia.unsqueeze(1).broadcast_to([P, sgt, 8]),
            )
            m1 = thp.tile([P, sgt, 8], F32, tag="m1")
            nc.vector.tensor_scalar(
                m1, a0, PI, -2.0 * PI,
                op0=mybir.AluOpType.is_gt, op1=mybir.AluOpType.mult,
            )
            a1 = thp.tile([P, sgt, 8], F32, tag="a1")
            nc.vector.tensor_add(a1, a0, m1)
            m2 = thp.tile([P, sgt, 8], F32, tag="m2")
            nc.vector.tensor_scalar(
                m2, a1, -PI, 2.0 * PI,
                op0=mybir.AluOpType.is_lt, op1=mybir.AluOpType.mult,
            )
            thin = thp.tile([P, sgt, 8], FP16, tag="thin")
            nc.vector.tensor_add(thin, a1, m2)

            # v = [1, sin, cos] per wire: vv[p, g, m, w]
            vv = vvp.tile([P, sgt, 3, 4], FP16)
            nc.gpsimd.memset(vv[:, :, 0, :], 1.0)
            nc.scalar.activation(
                vv[:, :, 1:3, :],
                thin.rearrange("p g (c w) -> p g c w", c=2),
                mybir.ActivationFunctionType.Sin,
            )

            # w01[m0,m1] = v0[m0]*v1[m1]; w23[m2,m3] = v2[m2]*v3[m3]
            wpair = wp.tile([P, sgt, 2, 3, 3], FP16)
            nc.vector.tensor_mul(
                wpair[:, :, 0],
                vv[:, :, :, 0].unsqueeze(3).broadcast_to([P, sgt, 3, 3]),
                vv[:, :, :, 1].unsqueeze(2).broadcast_to([P, sgt, 3, 3]),
            )
            nc.vector.tensor_mul(
                wpair[:, :, 1],
                vv[:, :, :, 2].unsqueeze(3).broadcast_to([P, sgt, 3, 3]),
                vv[:, :, :, 3].unsqueeze(2).broadcast_to([P, sgt, 3, 3]),
            )
            w01 = wpair[:, :, 0].rearrange("p g a b -> p g (a b)")
            w23 = wpair[:, :, 1].rearrange("p g a b -> p g (a b)")

            # tq[p,g,km,m23] = w23[m23] * T[km, m23]  ((k,m01) merged -> 18)
            tq = tqp.tile([P, sgt, 18, 9], FP16)
            nc.vector.tensor_mul(
                tq,
                w23.unsqueeze(2).broadcast_to([P, sgt, 18, 9]),
                tco[:, 0:162].rearrange("p (km b) -> p km b", b=9)
                   .unsqueeze(1).broadcast_to([P, sgt, 18, 9]),
            )
            qk = qkp.tile([P, sgt, 18], FP16, tag="qk")
            with nc.allow_low_precision(reason="fp16 qk; gate is 2e-2"):
                nc.vector.tensor_reduce(
                    qk, tq, axis=mybir.AxisListType.X, op=mybir.AluOpType.add
                )
            sk = qkp.tile([P, sgt, 2, 9], F32, tag="sk")
            nc.vector.tensor_mul(
                sk,
                qk.rearrange("p g (k m) -> p g k m", m=9),
                w01.unsqueeze(2).broadcast_to([P, sgt, 2, 9]),
            )
            res = rp.tile([P, sgt, 2], F32)
            nc.vector.tensor_reduce(
                res, sk, axis=mybir.AxisListType.X, op=mybir.AluOpType.add
            )
            nc.sync.dma_start(out_d[:, off:off + sgt, :], res)
            off += sgt

    nc.finalize()
    return nc


_NC_CACHE = {}


def _get_nc(rows=ROWS):
    if rows not in _NC_CACHE:
        _NC_CACHE[rows] = build_bass(rows=rows)
    return _NC_CACHE[rows]


def _host_consts(pre_w, pre_b, q_weights, post_w, post_b):
    pre_w = np.asarray(pre_w, dtype=np.float32)
    wdup = np.zeros((P, 32), dtype=np.float16)
    for k in range(4):
        blk = pre_w.T[P * k:P * (k + 1)].astype(np.float16)  # [128, 4]
        wdup[:, 8 * k:8 * k + 4] = blk
        wdup[:, 8 * k + 4:8 * k + 8] = blk
    pb = np.asarray(pre_b, np.float64)
    b8 = np.concatenate([pb, pb + 0.5 * np.pi]).astype(np.float32)
    bias2 = np.broadcast_to(b8, (P, 8)).copy()
    T = _build_T(
        np.asarray(q_weights, np.float64),
        np.asarray(post_w, np.float64),
        np.asarray(post_b, np.float64),
    )  # [2, 81] f32
    tco = np.broadcast_to(T.reshape(162), (P, 162)).astype(np.float16).copy()
    return {
        "wdup": np.ascontiguousarray(wdup),
        "bias2": np.ascontiguousarray(bias2),
        "tcoef": np.ascontiguousarray(tco),
    }


def _pack_x(x):
    """x [ROWS, F] f32 -> flat fp16, per-slab [P, 4, SLAB*P] packs with
    pack[p, k, b] = x[slab_row0 + b, 128*k + p]."""
    rows = x.shape[0]
    h = x.astype(np.float16)
    gb = SLAB * P
    parts = []
    for r0 in range(0, rows, gb):
        blk = h[r0:r0 + gb].reshape(gb, 4, P).transpose(2, 1, 0)
        parts.append(np.ascontiguousarray(blk).reshape(-1))
    return np.concatenate(parts)


def run(input_features, pre_w, pre_b, q_weights, post_w, post_b, **spmd_kwargs):
    x = np.asarray(input_features, dtype=np.float32)
    assert x.shape == (B_TOTAL, F_IN), x.shape
    consts = _host_consts(pre_w, pre_b, q_weights, post_w, post_b)
    in_maps = []
    for c in range(N_CORES):
        in_maps.append(dict(consts, xtp=_pack_x(x[c * ROWS:(c + 1) * ROWS])))
    nc = _get_nc()
    r = run_bass_kernel_spmd(nc, in_maps, core_ids=list(range(N_CORES)), **spmd_kwargs)
    # out_dev[p, t, k] -> out[t*128 + p, k]
    out = np.concatenate(
        [r.results[c]["out"].transpose(1, 0, 2).reshape(ROWS, 2) for c in range(N_CORES)],
        axis=0,
    )
    return out.astype(np.float32), r


def kernel(input_features, pre_w, pre_b, q_weights, post_w, post_b):
    out, _ = run(input_features, pre_w, pre_b, q_weights, post_w, post_b)
    return out


# revision 5
# speedup vs baseline: 1.3785x; 1.0100x over previous
"""DressedQuantumNet on 8 TRN2 NeuronCores (pure data parallel).

Math: pre-net angles th = X @ pre_w.T + pre_b.  The quantum circuit after
the batch-dependent RY(th) layer is a FIXED unitary V (it only depends on
q_weights), and the initial state is the product state
psi = kron_w [a_w, b_w] with a=(cos(th/2)-sin(th/2))/sqrt(2),
b=(cos(th/2)+sin(th/2))/sqrt(2) (real).  Hence

  <Z_w> = psi^T Re(V^H Z_w V) psi
  out_k = psi^T C_k psi + post_b_k,   C_k = sum_w post_w[k,w] Re(V^H Z_w V)

and since (u u^T) is affine in (sin th, cos th), the whole quadratic form
collapses to an 81-coefficient multilinear polynomial in
v_w = [1, sin th_w, cos th_w]:

  out_k = sum_{m in 3^4} T_k[m] * prod_w v_w[m_w]

T_k is precomputed on host (tiny), so the device only computes the big
[B,512]@[512,4] matmul, sin/cos, and a few batched elementwise products.

Device layout (per core, batch on SBUF partitions):
  - X shipped fp16 (host-rounded; rel err ~8e-4 vs the 2e-2 gate), slabs
    of 8 row-tiles DMA'd on alternating HWDGE rings (sync/scalar).
  - per tile: ones-row matmul seeds PSUM with [pre_b | pre_b + pi/2],
    then 4 fp16 matmuls vs duplicated W cols -> ang[p, t, 8] = theta and
    theta + pi/2 in one PSUM tile.
  - one conditional range-wrap into [-pi, pi] on DVE (|theta| < 4.6, so
    one wrap is exact), one Sin LUT pass -> sin/cos fp16.
  - kron products + T contraction in fp16 (2x DVE rate), outer products
    and the small sk multiply on the otherwise-idle Pool (gpsimd) engine.
"""

from contextlib import ExitStack

import numpy as np

import concourse.bass as bass
import concourse.bacc as bacc_mod
import concourse.mybir as mybir
from concourse.bass_utils import run_bass_kernel_spmd
from concourse.tile import TileContext

N_CORES = 8
B_TOTAL = 65536
F_IN = 512
ROWS = B_TOTAL // N_CORES   # 8192 rows per core
P = 128
N_TILES = ROWS // P         # 64 row-tiles
SLAB = 8                    # row-tiles per input DMA (1 MiB fp16)
SUPER = 2                   # slabs per quantum-stage group

F32 = mybir.dt.float32
FP16 = mybir.dt.float16
PI = float(np.pi)

N_QUBITS, VAR_DEPTH = 4, 3


# ----------------------------------------------------------------- host math
def _gate_1q(g, w):
    ops = [np.eye(2, dtype=complex)] * N_QUBITS
    ops[w] = g
    U = ops[0]
    for i in range(1, N_QUBITS):
        U = np.kron(U, ops[i])
    return U


def _bit(i, w):  # wire 0 = most significant
    return (i >> (N_QUBITS - 1 - w)) & 1


def _cnot(c, t):
    M = np.zeros((16, 16), dtype=complex)
    for i in range(16):
        j = i ^ (1 << (N_QUBITS - 1 - t)) if _bit(i, c) else i
        M[j, i] = 1.0
    return M


def _ry(theta):
    c, s = np.cos(theta / 2), np.sin(theta / 2)
    return np.array([[c, -s], [s, c]], dtype=complex)


def _rz(theta):
    ph = np.exp(1j * theta / 2)
    return np.array([[np.conj(ph), 0], [0, ph]], dtype=complex)


def _fixed_unitary(qw):
    V = np.eye(16, dtype=complex)

    def app(Gm):
        nonlocal V
        V = Gm @ V

    def entangle():
        app(_cnot(0, 1)); app(_cnot(2, 3)); app(_cnot(1, 2))

    for k in range(VAR_DEPTH):
        entangle()
        for w in range(N_QUBITS):
            app(_gate_1q(_ry(qw[k, w]), w))
        for w in range(N_QUBITS):
            app(_gate_1q(_rz(qw[k, w]), w))
    for k in range(VAR_DEPTH):
        entangle()
        for w in range(N_QUBITS):
            app(_gate_1q(_ry(qw[k, w]), w))
        for w in range(N_QUBITS):
            app(_gate_1q(_rz(qw[3 + k, w]), w))
    entangle()
    return V


def _build_T(q_weights, post_w, post_b):
    """[2, 81] coefficients; post_b folded into the constant term."""
    V = _fixed_unitary(np.asarray(q_weights, dtype=np.float64))
    E = np.zeros((3, 2, 2))
    E[0] = [[0.5, 0.0], [0.0, 0.5]]
    E[1] = [[-0.5, 0.0], [0.0, 0.5]]
    E[2] = [[0.0, 0.5], [0.5, 0.0]]
    Ts = []
    for k in range(2):
        C = np.zeros((16, 16), dtype=complex)
        for w in range(N_QUBITS):
            z = np.array([1.0 - 2.0 * _bit(i, w) for i in range(16)])
            C += post_w[k, w] * (V.conj().T @ np.diag(z) @ V)
        A = C.real.reshape([2] * 8)
        T = np.einsum("abcdefgh,iae,jbf,kcg,ldh->ijkl", A, E, E, E, E)
        T = T.reshape(81).copy()
        T[0] += post_b[k]
        Ts.append(T)
    return np.stack(Ts).astype(np.float32)  # [2, 81]


# ------------------------------------------------------------- device kernel
def build_bass(rows=ROWS):
    n_tiles = rows // P
    n_slabs = n_tiles // SLAB
    assert n_tiles % SLAB == 0

    nc = bacc_mod.Bacc(None, target_bir_lowering=False)
    # host-packed flat: per slab [P, 4, SLAB*P], pack[p,k,b] = X[b, 128k+p]
    x_d = nc.dram_tensor("xtp", [rows * 4 * P], FP16, kind="ExternalInput")
    w_d = nc.dram_tensor("wdup", [P, 32], FP16, kind="ExternalInput")
    pb_d = nc.dram_tensor("bias2", [P, 8], F32, kind="ExternalInput")
    tc_d = nc.dram_tensor("tcoef", [P, 162], FP16, kind="ExternalInput")
    # out_dev[p, t, k] = out[t*128 + p, k]; host unscrambles
    out_d = nc.dram_tensor("out", [P, n_tiles, 2], F32, kind="ExternalOutput")

    with TileContext(nc) as tc, ExitStack() as ctx:
        const = ctx.enter_context(tc.tile_pool(name="const", bufs=1))
        wdup = const.tile([P, 32], FP16)
        nc.scalar.dma_start(wdup, w_d[:])
        bia = const.tile([P, 8], F32)
        nc.scalar.dma_start(bia, pb_d[:])
        tco = const.tile([P, 162], FP16)
        nc.scalar.dma_start(tco, tc_d[:])
        zt = const.tile([P, 1], F32)
        nc.vector.memset(zt, 0.0)
        warm = const.tile([P, 1], F32)
        # preload the Sin table set under the first input DMA
        nc.scalar.activation(warm, zt, mybir.ActivationFunctionType.Sin)

        xp = ctx.enter_context(tc.tile_pool(name="xin", bufs=6))
        angp = ctx.enter_context(tc.tile_pool(name="ang", bufs=3, space="PSUM"))
        thp = ctx.enter_context(tc.tile_pool(name="th", bufs=3))
        vvp = ctx.enter_context(tc.tile_pool(name="vv", bufs=3))
        wp = ctx.enter_context(tc.tile_pool(name="wpair", bufs=3))
        tqp = ctx.enter_context(tc.tile_pool(name="tq", bufs=3))
        qkp = ctx.enter_context(tc.tile_pool(name="qk", bufs=3))
        rp = ctx.enter_context(tc.tile_pool(name="res", bufs=2))

        dma_engines = [nc.sync, nc.scalar]
        n_supers = (n_slabs + SUPER - 1) // SUPER
        state = {}

        def stage_a(si):
            """DMA + matmul + wrap + Sin for super si."""
            s0 = si * SUPER
            sl_cnt = min(SUPER, n_slabs - s0)
            sgt = sl_cnt * SLAB
            x_tiles = []
            for j in range(sl_cnt):
                slab_idx = s0 + j
                gb = SLAB * P
                base = slab_idx * P * 4 * gb
                xt = xp.tile([P, 4, gb], FP16, tag="x")
                dma_engines[slab_idx % 2].dma_start(
                    xt,
                    x_d[base:base + P * 4 * gb].rearrange(
                        "(p k b) -> p k b", p=P, k=4),
                )
                x_tiles.append(xt)

            # ang[p, t, 0:4] = theta_raw; ang[p, t, 4:8] = theta_raw (dup cols)
            ang = angp.tile([P, sgt, 8], F32)
            for ti in range(sgt):
                sl, tloc = divmod(ti, SLAB)
                bs = tloc * P
                for k in range(4):
                    nc.tensor.matmul(
                        ang[:, ti, :],
                        x_tiles[sl][:, k, bs:bs + P],
                        wdup[:, 8 * k:8 * k + 8],
                        start=(k == 0), stop=(k == 3),
                    )

            # bias, then range-wrap into [-pi, pi] (|theta| < 4.6, one wrap
            # is exact; the cos half only ever needs the positive wrap)
            a0 = thp.tile([P, sgt, 8], F32, tag="a0")
            nc.vector.tensor_add(
                a0, ang,
                bia.unsqueeze(1).broadcast_to([P, sgt, 8]),
            )
            m1 = thp.tile([P, sgt, 8], F32, tag="m1")
            nc.vector.tensor_scalar(
                m1, a0, PI, -2.0 * PI,
                op0=mybir.AluOpType.is_gt, op1=mybir.AluOpType.mult,
            )
            a1 = thp.tile([P, sgt, 8], F32, tag="a1")
            nc.vector.tensor_add(a1, a0, m1)
            m2 = thp.tile([P, sgt, 8], F32, tag="m2")
            nc.vector.tensor_scalar(
                m2, a1, -PI, 2.0 * PI,
                op0=mybir.AluOpType.is_lt, op1=mybir.AluOpType.mult,
            )
            thin = thp.tile([P, sgt, 8], FP16, tag="thin")
            nc.vector.tensor_add(thin, a1, m2)

            # v = [1, sin, cos] per wire: vv[p, g, m, w]
            vv = vvp.tile([P, sgt, 3, 4], FP16)
            nc.gpsimd.memset(vv[:, :, 0, :], 1.0)
            nc.scalar.activation(
                vv[:, :, 1:3, :],
                thin.rearrange("p g (c w) -> p g c w", c=2),
                mybir.ActivationFunctionType.Sin,
            )
            state[si] = (vv, sgt)

        def stage_b(si, off):
            """kron products + T contraction for super si."""
            vv, sgt = state.pop(si)
            # w01[m0,m1] = v0[m0]*v1[m1]; w23[m2,m3] = v2[m2]*v3[m3]
            wpair = wp.tile([P, sgt, 2, 3, 3], FP16)
            nc.vector.tensor_mul(
                wpair[:, :, 0],
                vv[:, :, :, 0].unsqueeze(3).broadcast_to([P, sgt, 3, 3]),
                vv[:, :, :, 1].unsqueeze(2).broadcast_to([P, sgt, 3, 3]),
            )
            nc.vector.tensor_mul(
                wpair[:, :, 1],
                vv[:, :, :, 2].unsqueeze(3).broadcast_to([P, sgt, 3, 3]),
                vv[:, :, :, 3].unsqueeze(2).broadcast_to([P, sgt, 3, 3]),
            )
            w01 = wpair[:, :, 0].rearrange("p g a b -> p g (a b)")
            w23 = wpair[:, :, 1].rearrange("p g a b -> p g (a b)")

            # tq[p,g,km,m23] = w23[m23] * T[km, m23]  ((k,m01) merged -> 18)
            tq = tqp.tile([P, sgt, 18, 9], FP16)
            nc.vector.tensor_mul(
                tq,
                w23.unsqueeze(2).broadcast_to([P, sgt, 18, 9]),
                tco[:, 0:162].rearrange("p (km b) -> p km b", b=9)
                   .unsqueeze(1).broadcast_to([P, sgt, 18, 9]),
            )
            qk = qkp.tile([P, sgt, 18], FP16, tag="qk")
            with nc.allow_low_precision(reason="fp16 qk; gate is 2e-2"):
                nc.vector.tensor_reduce(
                    qk, tq, axis=mybir.AxisListType.X, op=mybir.AluOpType.add
                )
            sk = qkp.tile([P, sgt, 2, 9], F32, tag="sk")
            nc.vector.tensor_mul(
                sk,
                qk.rearrange("p g (k m) -> p g k m", m=9),
                w01.unsqueeze(2).broadcast_to([P, sgt, 2, 9]),
            )
            res = rp.tile([P, sgt, 2], F32)
            nc.vector.tensor_reduce(
                res, sk, axis=mybir.AxisListType.X, op=mybir.AluOpType.add
            )
            nc.sync.dma_start(out_d[:, off:off + sgt, :], res)
            return off + sgt

        # software pipeline: run stage A two supers ahead of stage B so the
        # DVE queue always has independent work while ACT/PE catch up
        off = 0
        stage_a(0)
        if n_supers > 1:
            stage_a(1)
        for si in range(n_supers):
            off = stage_b(si, off)
            if si + 2 < n_supers:
                stage_a(si + 2)

    nc.finalize()
    return nc


_NC_CACHE = {}


def _get_nc(rows=ROWS):
    if rows not in _NC_CACHE:
        _NC_CACHE[rows] = build_bass(rows=rows)
    return _NC_CACHE[rows]


def _host_consts(pre_w, pre_b, q_weights, post_w, post_b):
    pre_w = np.asarray(pre_w, dtype=np.float32)
    wdup = np.zeros((P, 32), dtype=np.float16)
    for k in range(4):
        blk = pre_w.T[P * k:P * (k + 1)].astype(np.float16)  # [128, 4]
        wdup[:, 8 * k:8 * k + 4] = blk
        wdup[:, 8 * k + 4:8 * k + 8] = blk
    pb = np.asarray(pre_b, np.float64)
    b8 = np.concatenate([pb, pb + 0.5 * np.pi]).astype(np.float32)
    bias2 = np.broadcast_to(b8, (P, 8)).copy()
    T = _build_T(
        np.asarray(q_weights, np.float64),
        np.asarray(post_w, np.float64),
        np.asarray(post_b, np.float64),
    )  # [2, 81] f32
    tco = np.broadcast_to(T.reshape(162), (P, 162)).astype(np.float16).copy()
    return {
        "wdup": np.ascontiguousarray(wdup),
        "bias2": np.ascontiguousarray(bias2),
        "tcoef": np.ascontiguousarray(tco),
    }


def _pack_x(x):
    """x [ROWS, F] f32 -> flat fp16, per-slab [P, 4, SLAB*P] packs with
    pack[p, k, b] = x[slab_row0 + b, 128*k + p]."""
    rows = x.shape[0]
    h = x.astype(np.float16)
    gb = SLAB * P
    parts = []
    for r0 in range(0, rows, gb):
        blk = h[r0:r0 + gb].reshape(gb, 4, P).transpose(2, 1, 0)
        parts.append(np.ascontiguousarray(blk).reshape(-1))
    return np.concatenate(parts)


def run(input_features, pre_w, pre_b, q_weights, post_w, post_b, **spmd_kwargs):
    x = np.asarray(input_features, dtype=np.float32)
    assert x.shape == (B_TOTAL, F_IN), x.shape
    consts = _host_consts(pre_w, pre_b, q_weights, post_w, post_b)
    in_maps = []
    for c in range(N_CORES):
        in_maps.append(dict(consts, xtp=_pack_x(x[c * ROWS:(c + 1) * ROWS])))
    nc = _get_nc()
    r = run_bass_kernel_spmd(nc, in_maps, core_ids=list(range(N_CORES)), **spmd_kwargs)
    # out_dev[p, t, k] -> out[t*128 + p, k]
    out = np.concatenate(
        [r.results[c]["out"].transpose(1, 0, 2).reshape(ROWS, 2) for c in range(N_CORES)],
        axis=0,
    )
    return out.astype(np.float32), r


def kernel(input_features, pre_w, pre_b, q_weights, post_w, post_b):
    out, _ = run(input_features, pre_w, pre_b, q_weights, post_w, post_b)
    return out
